# revision 13
# baseline (speedup 1.0000x reference)
"""AttentionPooling (segment softmax-pool) Trainium2 Bass kernel, v2.

out[g, :] = sum_{i: batch[i]==g} softmax_within_segment(score)_i * x[i, :]
score_i = tanh(x_i @ W1 + b1) @ W2 + b2

Math notes:
- softmax is shift-invariant, so b2 and the per-segment max subtraction
  cancel exactly; we compute e_i = exp(s_i) with s_i = tanh(xW1+b1)@W2
  and normalize by the per-segment sum of e at the very end (on host).
- per-segment sums run on the TensorEngine with the one-hot weights as
  the MOVING operand and the x subtile as the STATIONARY operand:
  pooledT[f, g] += xn_sub[:, f].T @ ow[:, g].  The moving free size is
  GM (~65) instead of D+1 (257), cutting PE row count ~2x vs pooling
  with ow stationary.  The denominator rides a 1-row matmul per subtile
  (lhsT=ow, rhs=ones -> den[g, 0] += sum_i ow[i, g]).
- final normalize (divide by denominator, empty-segment guard) is done
  on the host: outputs are raw [128, 2*GM+1] f32 tiles (pooledT halves
  + denominator column).

Precision: BOTH x copies are fp8 e3m4 (12.85 MB/core HBM total): the
transposed copy feeds the score MLP (as in v1), the natural copy is the
pooling stationary.  fp8 pooling adds ~3-5e-3 rel err (validated ~1.2e-2
total vs the 2e-2 gate).

Engine balance per 512-node chunk (~49 chunks/core):
- PE: 2 W1 matmuls (512 fp8 rows ea) + 4 1-row score matmuls + 8 pool
  matmuls (GM rows ea) + 4 1-row denom matmuls ~ 1550 rows.
- ACT: one [128,512] tanh; exp is batched over Q=4 chunks into a single
  [128,16] instruction (ACT per-instruction overhead is ~200ns).
- one-hot builds (tensor_scalar is_equal*mult): 2 on DVE + 2 on GpSimd
  per chunk, halving the DVE bottleneck of v1.

Pipelining: stage skew keeps every cross-engine input produced at least
one full chunk before its consumer: W1/tanh(j), scores(j-1),
exp+ow(batch at batch boundary), pool(j-Q-1).

HBM traffic per core: 12.9 MB, one packed fp8 stream [128, T, 2048]
(per chunk: 1024B transposed + 1024B natural per partition), grouped
DMAs on the sync HWDGE ring in consumption order.  A byte-packed
preamble DMA carries weights/consts plus chunk 0 so a single completion
gates pipeline start.

Sharding: nodes split across 8 cores at segment boundaries (batch is
sorted); each core reduces its own segments; host normalizes and
concatenates.
"""

import sys

sys.path.insert(0, "/opt/trn_rl_repo")

import numpy as np
import ml_dtypes

import concourse.bass as bass
import concourse.tile as tile
from concourse import mybir
from concourse.bass_utils import run_bass_kernel_spmd

BF16 = ml_dtypes.bfloat16
E3M4 = ml_dtypes.float8_e3m4

N_CORES = 8
D = 256
H = 128  # hidden dim of the score MLP
C = 512  # nodes per chunk
SUB = C // 128
Q = 4  # chunks per exp batch
STEADY_GROUP = 6  # chunks per steady-state DMA group


def _split_multiwait(nc):
    """Split multi-wait instructions for this walrus build.

    This neuronxcc/walrus rejects more than one sync-wait command per
    instruction ("Too many sync wait commands"), but tile emits 2-3 waits
    on compute/DMA instructions and many on the final Drain. Hoist the
    extra waits onto preceding InstEventSemaphore instructions (the native
    sequencer wait primitive, 2 waits each) on the same engine. Engine
    program order makes this equivalent: the stream blocks on the EVSEM
    waits, then on the instruction's remaining wait.
    """
    for bb in nc.main_func.blocks:
        new = []
        for ins in bb.instructions:
            w = (
                list(ins.sync_info.on_wait)
                if (ins.sync_info and ins.sync_info.on_wait)
                else []
            )
            if len(w) > 1:
                extras = w[:-1]
                for i in range(0, len(extras), 2):
                    ev = mybir.InstEventSemaphore(
                        name=nc.get_next_instruction_name(),
                        engine=ins.engine,
                        sync_info=mybir.SyncInfo(
                            on_wait=extras[i : i + 2], on_update=[]
                        ),
                    )
                    nc.register_instruction(ev)
                    new.append(ev)
                ins.sync_info.on_wait = [w[-1]]
            new.append(ins)
        bb.instructions[:] = new


def _group_plan(T):
    """DMA grouping: small leading groups to prime the pipeline, then big."""
    plan = []
    t = 0
    for n in (1, 1, 2, 4):
        if t >= T:
            break
        n = min(n, T - t)
        plan.append((t, t + n))
        t += n
    while t < T:
        n = min(STEADY_GROUP, T - t)
        plan.append((t, t + n))
        t += n
    return plan


def _build_program(T, GM):
    """Build the SPMD Bass program: T chunks of C nodes, GM local segments."""
    f32 = mybir.dt.float32
    bf16 = mybir.dt.bfloat16
    fp8 = mybir.dt.float8e3
    T4 = T * SUB
    CB = 2 * C + SUB * D  # fp8 bytes per chunk per partition (xt | xn)

    nc = bass.Bass(trn_type="TRN2")
    xp = nc.dram_tensor("xp", [128, T, CB], fp8, kind="ExternalInput")
    # byte-packed preamble, one DMA gating the pipeline start:
    # w1a|w1b|w2|ones|mcf(f32: b1|bl|iota)
    NF = 1 + T4 + GM
    PRE = 516 + 4 * NF
    pre = nc.dram_tensor("pre", [128, PRE], fp8, kind="ExternalInput")
    out = nc.dram_tensor("out", [128, 2 * GM + 1], f32, kind="ExternalOutput")

    Exp = mybir.ActivationFunctionType.Exp
    Tanh = mybir.ActivationFunctionType.Tanh

    plan = _group_plan(T)

    with tile.TileContext(nc) as tc:
        with (
            tc.tile_pool(name="const", bufs=1) as const,
            tc.tile_pool(name="thtp", bufs=3) as thtp,
            tc.tile_pool(name="owp", bufs=10) as owp,
            tc.tile_pool(name="etp", bufs=3) as etp,
            tc.tile_pool(name="outp", bufs=1) as outp,
            tc.tile_pool(name="hps", bufs=3, space="PSUM") as hps,
            tc.tile_pool(name="sps", bufs=2, space="PSUM") as sps,
            tc.tile_pool(name="accp", bufs=1, space="PSUM") as accp,
        ):
            xpg = [
                const.tile([128, t1 - t0, CB], fp8, name=f"xpg{gi}")
                for gi, (t0, t1) in enumerate(plan)
            ]
            gidx = []
            for gi, (t0, t1) in enumerate(plan):
                for lt in range(t1 - t0):
                    gidx.append((gi, lt))

            # all input loads on the sync HWDGE ring, in consumption order
            pres = const.tile([128, PRE], fp8)
            nc.sync.dma_start(out=pres, in_=pre[:, :])
            for gi, (t0, t1) in enumerate(plan):
                nc.sync.dma_start(out=xpg[gi], in_=xp[:, t0:t1])

            w1a = pres[:, 0:256].bitcast(bf16)
            w1b = pres[:, 256:512].bitcast(bf16)
            w2sb = pres[:, 512:514].bitcast(bf16)
            onesb = pres[:, 514:516].bitcast(bf16)
            mcfv = pres[:, 516 : 516 + 4 * NF].bitcast(f32)
            b1sb = mcfv[:, 0:1]
            blsb = mcfv[:, 1 : 1 + T4]
            iosb = mcfv[:, 1 + T4 : 1 + T4 + GM]

            def xt_half(c, h):
                gi, lt = gidx[c]
                return xpg[gi][:, lt, h * C : (h + 1) * C]

            def xn_half(c, a, f):
                o = 2 * C + a * D + f * 128
                gi, lt = gidx[c]
                return xpg[gi][:, lt, o : o + 128]

            # persistent PSUM accumulator spanning the whole kernel:
            # pooledT feature halves + denominator column, packed into one
            # bank: acc = [ptA (GM) | ptB (GM) | den (1)]
            acc = accp.tile([128, 2 * GM + 1], f32)
            pta = acc[:, 0:GM]
            ptb = acc[:, GM : 2 * GM]
            den = acc[0:GM, 2 * GM : 2 * GM + 1]

            # zero tile opening each accumulation group: the hardware
            # mis-processes the first matmul of a long accumulation group
            # (observed as garbage in its first columns), so open the
            # groups with an all-zero matmul and accumulate real data
            # with start=False.
            zt = const.tile([128, 128], bf16)
            nc.vector.memset(zt, 0.0)
            nc.tensor.matmul(
                pta, lhsT=zt, rhs=zt[:, 0:GM], start=True, stop=False,
                skip_group_check=True,
            )
            nc.tensor.matmul(
                ptb, lhsT=zt, rhs=zt[:, 0:GM], start=True, stop=False,
                skip_group_check=True,
            )
            nc.tensor.matmul(
                den, lhsT=zt[:, 0:GM], rhs=zt[:, 0:1], start=True, stop=False,
                skip_group_check=True,
            )

            NB = (T + Q - 1) // Q  # number of exp batches
            hp_t = [None] * T
            tht_t = [None] * T
            spb_t = [None] * NB
            etb_t = [None] * NB
            owt_t = [None] * T

            for j in range(T + Q + 1):
                # stage A: W1 matmul + tanh for chunk j
                if j < T:
                    if j % Q == 0:
                        spb_t[j // Q] = sps.tile([128, Q * SUB], f32, name="spb")
                    hp = hps.tile([H, C], f32)
                    nc.tensor.matmul(
                        hp, lhsT=w1a, rhs=xt_half(j, 0), start=True, stop=False
                    )
                    nc.tensor.matmul(
                        hp, lhsT=w1b, rhs=xt_half(j, 1), start=False, stop=True
                    )
                    tht = thtp.tile([H, C], bf16)
                    nc.scalar.activation(tht, hp, Tanh, bias=b1sb)
                    hp_t[j], tht_t[j] = hp, tht

                # stage B: score matmuls for chunk j-1 into its batch slot
                if 0 <= j - 1 < T:
                    jb = j - 1
                    b = jb // Q
                    spb = spb_t[b]
                    tht = tht_t[jb]
                    col0 = (jb % Q) * SUB
                    for a in range(SUB):
                        nc.tensor.matmul(
                            spb[:, col0 + a : col0 + a + 1],
                            lhsT=tht[:, a * 128 : (a + 1) * 128],
                            rhs=w2sb,
                            start=True,
                            stop=True,
                            skip_group_check=True,
                        )
                    # stage C: batch complete -> one exp + one-hot builds
                    if jb % Q == Q - 1 or jb == T - 1:
                        qc = (jb % Q) + 1  # chunks in this batch
                        etb = etp.tile([128, Q * SUB], f32)
                        nc.scalar.activation(
                            etb[:, 0 : qc * SUB], spb[:, 0 : qc * SUB], Exp
                        )
                        etb_t[b] = etb
                        for c in range(b * Q, b * Q + qc):
                            owt = owp.tile([128, SUB, GM], bf16)
                            cc = (c % Q) * SUB
                            for a in range(SUB):
                                eng = nc.vector if a < 2 else nc.gpsimd
                                eng.tensor_scalar(
                                    out=owt[:, a, :],
                                    in0=iosb,
                                    scalar1=blsb[:, c * SUB + a : c * SUB + a + 1],
                                    scalar2=etb[:, cc + a : cc + a + 1],
                                    op0=mybir.AluOpType.is_equal,
                                    op1=mybir.AluOpType.mult,
                                )
                            owt_t[c] = owt

                # stage E: pooling matmuls for chunk j-Q-1 (ow ready at least
                # one iteration ago, so the PE never waits here)
                jc = j - Q - 1
                if 0 <= jc < T:
                    owt = owt_t[jc]
                    for a in range(SUB):
                        first = False
                        last = jc == T - 1 and a == SUB - 1
                        nc.tensor.matmul(
                            pta,
                            lhsT=xn_half(jc, a, 0),
                            rhs=owt[:, a, :],
                            start=first,
                            stop=last,
                            skip_group_check=True,
                        )
                        nc.tensor.matmul(
                            ptb,
                            lhsT=xn_half(jc, a, 1),
                            rhs=owt[:, a, :],
                            start=first,
                            stop=last,
                            skip_group_check=True,
                        )
                        nc.tensor.matmul(
                            den,
                            lhsT=owt[:, a, :],
                            rhs=onesb,
                            start=first,
                            stop=last,
                            skip_group_check=True,
                        )

            # stage F: stage raw accumulators to SBUF and DMA out;
            # normalization happens on the host. Columns of partitions
            # >= GM in the den column are uninitialized psum; host ignores.
            ot = outp.tile([128, 2 * GM + 1], f32)
            nc.scalar.copy(ot, acc)
            # scalar HWDGE ring: empty at this point, while the sync ring
            # may still be retiring trailing input groups
            nc.scalar.dma_start(out=out[:, :], in_=ot)

    _split_multiwait(nc)
    return nc


def _prepare(inputs):
    """Host-side sharding and input staging. Returns (meta, in_maps)."""
    x = np.asarray(inputs["x"], dtype=np.float32)
    batch = np.asarray(inputs["batch"]).astype(np.int64)
    W1 = np.asarray(inputs["W1"], dtype=np.float32)
    b1 = np.asarray(inputs["b1"], dtype=np.float32)
    W2 = np.asarray(inputs["W2"], dtype=np.float32)

    n, d = x.shape
    assert d == D
    G = 512
    seg_ptr = np.searchsorted(batch, np.arange(G + 1))  # [G+1], seg g rows

    # split at segment boundaries, balancing rows
    targets = (np.arange(N_CORES + 1) * n) // N_CORES
    g_bounds = np.zeros(N_CORES + 1, dtype=np.int64)
    g_bounds[N_CORES] = G
    for c in range(1, N_CORES):
        g = int(np.argmin(np.abs(seg_ptr.astype(np.int64) - targets[c])))
        g_bounds[c] = max(g, g_bounds[c - 1])
    row_bounds = seg_ptr[g_bounds]

    rows = np.diff(row_bounds)
    segs = np.diff(g_bounds)
    GM = int(segs.max())
    assert GM <= 128, f"too many segments on one core: {GM}"
    T = int(-(-int(rows.max()) // C))
    R = T * C
    T4 = T * SUB
    CB = 2 * C + SUB * D

    # bf16 consts: W1 halves | W2 | ones, byte view for the packed preamble
    mcb = np.zeros((128, 2 * H + 2), dtype=BF16)
    mcb[:, 0:H] = W1[0:128].astype(BF16)
    mcb[:, H : 2 * H] = W1[128:256].astype(BF16)
    mcb[:, 2 * H] = W2[:, 0].astype(BF16)
    mcb[:, 2 * H + 1] = np.float32(1.0)
    mcb_bytes = np.ascontiguousarray(mcb).view(np.uint8)  # [128, 516]

    iota = np.arange(GM, dtype=np.float32)

    in_maps = []
    for c in range(N_CORES):
        r0, r1 = int(row_bounds[c]), int(row_bounds[c + 1])
        g0, g1 = int(g_bounds[c]), int(g_bounds[c + 1])
        nr = r1 - r0
        xpad = np.zeros((R, D), dtype=np.float32)
        xpad[:nr] = x[r0:r1]
        xe = xpad.astype(E3M4)
        # transposed layout: [128, T, 2, C] fp8 e3m4
        xth = np.ascontiguousarray(xe.reshape(T, C, 2, 128).transpose(3, 0, 2, 1))
        # natural layout: [128, T, SUB, D] fp8 e3m4
        xnh = np.ascontiguousarray(xe.reshape(T, SUB, 128, D).transpose(2, 0, 1, 3))
        # packed per-chunk stream: [128, T, 2C + SUB*D]
        xph = np.concatenate(
            [xth.reshape(128, T, 2 * C), xnh.reshape(128, T, SUB * D)], axis=2
        )
        blp = np.full(R, -1.0, dtype=np.float32)
        blp[:nr] = (batch[r0:r1] - g0).astype(np.float32)
        # merged f32 consts: [128, 1+T4+GM] = b1 | bl | iota
        mcf = np.zeros((128, 1 + T4 + GM), dtype=np.float32)
        mcf[:, 0] = b1
        mcf[:, 1 : 1 + T4] = blp.reshape(T4, 128).T
        mcf[:, 1 + T4 : 1 + T4 + GM] = iota[None, :]
        # byte-packed preamble: bf16 consts + f32 consts
        pre = np.concatenate(
            [mcb_bytes, np.ascontiguousarray(mcf).view(np.uint8)],
            axis=1,
        ).view(E3M4)
        in_maps.append({"xp": xph, "pre": pre})

    meta = {
        "T": T,
        "GM": GM,
        "g_bounds": g_bounds,
        "G": G,
        "n": n,
    }
    return meta, in_maps


def _gather(meta, res):
    G = meta["G"]
    GM = meta["GM"]
    g_bounds = meta["g_bounds"]
    full = np.zeros((G, D), dtype=np.float32)
    for c in range(N_CORES):
        g0, g1 = int(g_bounds[c]), int(g_bounds[c + 1])
        if g1 <= g0:
            continue
        gm = g1 - g0
        o = res.results[c]["out"]  # [128, 2*GM+1] f32
        pt = np.concatenate([o[:, 0:GM], o[:, GM : 2 * GM]], axis=0)  # [256, GM]
        dn = o[0:GM, 2 * GM]
        dn = np.where(dn > 0, dn, 1.0)
        full[g0:g1] = (pt[:, :gm] / dn[None, :gm]).T
    return full


def _sane(full):
    # output rows are convex combinations of x rows (|x| < ~6); a device
    # glitch shows up as a huge value or NaN.
    return bool(np.isfinite(full).all() and np.abs(full).max() < 64.0)


def _run(inputs, trace=False):
    meta, in_maps = _prepare(inputs)
    nc = _build_program(meta["T"], meta["GM"])
    try:
        res = run_bass_kernel_spmd(nc, in_maps, list(range(N_CORES)), trace=trace)
        full = _gather(meta, res)
        if not _sane(full):
            raise RuntimeError("insane output, retrying once")
    except Exception:
        # transient device failures (e.g. NRT_EXEC_UNIT_UNRECOVERABLE) happen;
        # one rebuild+retry
        nc = _build_program(meta["T"], meta["GM"])
        res = run_bass_kernel_spmd(nc, in_maps, list(range(N_CORES)), trace=trace)
        full = _gather(meta, res)
    return full, res


def kernel(**inputs) -> np.ndarray:
    out, _ = _run(inputs, trace=False)
    return out


def kernel_traced(**inputs):
    """Returns (output, BassKernelResults with exec_time_ns/profile)."""
    out, res = _run(inputs, trace=True)
    return out, res


# revision 15
# speedup vs baseline: 1.8431x; 1.8431x over previous
"""AttentionPooling (segment softmax-pool) Trainium2 Bass kernel, v3.

out[g, :] = sum_{i: batch[i]==g} softmax_within_segment(score)_i * x[i, :]
score_i = tanh(x_i @ W1 + b1) @ W2 + b2

Math notes:
- softmax is shift-invariant, so b2 and the per-segment max subtraction
  cancel exactly; we compute e_i = exp(s_i) with s_i = tanh(xW1+b1)@W2
  and normalize by the per-segment sum of e at the very end (column D of
  the PSUM accumulator, via the ones column appended to x).
- per-segment sums run on the TensorEngine: for each 128-node subtile,
  the one-hot matrix ow[i, g] = e_i * (batch_local[i] == g) is the
  stationary operand and [x | 1] (fp8) the moving one; fp8 moving double
  pumps on HW (~0.21 ns/row), so each 257-row pool matmul is ~120 ns.
  Accumulation alternates between TWO psum banks (pchA even subtiles,
  pchB odd) to hide the per-bank accumulate turnaround; the banks are
  summed at the end.

Precision: both x copies ride fp8 e3m4 (the score MLP consumes the
transposed copy, pooling the natural copy + ones column). Validated
rel_err ~1.5e-2 vs the f32 reference (gate 2e-2).

Engine-level changes vs v1 (82.5us):
- DMA: 19.9 MB -> 14.8 MB (natural copy fp8 instead of bf16, plus a
  preloaded fp8 one-hot pattern per subtile).
- DVE: the 4 tensor_scalar one-hot builds per chunk (~300 ns each,
  instruction-overhead-bound) become ONE tensor_tensor: preloaded
  one-hot (bl_i == g) times exp(s) broadcast via a stride-0 AP.
- ACT: exp is batched over Q=4 chunks into one [128,16] instruction.
- PE: pool matmuls consume fp8 (2x moving rate).

Pipelining: stage skew keeps cross-engine inputs produced at least one
full chunk ahead: W1/tanh(j), scores(j-1), exp+ow at batch boundaries,
pool(j-Q-1).

HBM traffic per core: one packed fp8 stream [128, T, 2312] (per chunk
and partition: 1024B transposed x | 1028B natural x+ones | 260B
one-hot), grouped DMAs on the sync HWDGE ring in consumption order. A
byte-packed preamble DMA carries the weights/consts so a single
completion gates pipeline start.

Sharding: nodes split across 8 cores at segment boundaries (batch is
sorted); each core reduces its own segments; host concatenates the
per-core [G_c, D] outputs.
"""

import sys

sys.path.insert(0, "/opt/trn_rl_repo")

import numpy as np
import ml_dtypes

import concourse.bass as bass
import concourse.tile as tile
from concourse import mybir
from concourse.bass_utils import run_bass_kernel_spmd

BF16 = ml_dtypes.bfloat16
E3M4 = ml_dtypes.float8_e3m4

N_CORES = 8
D = 256
H = 128  # hidden dim of the score MLP
C = 512  # nodes per chunk
SUB = C // 128
Q = 4  # chunks per exp batch
STEADY_GROUP = 6  # chunks per steady-state DMA group


def _split_multiwait(nc):
    """Split multi-wait instructions for this walrus build.

    This neuronxcc/walrus rejects more than one sync-wait command per
    instruction ("Too many sync wait commands"), but tile emits 2-3 waits
    on compute/DMA instructions and many on the final Drain. Hoist the
    extra waits onto preceding InstEventSemaphore instructions (the native
    sequencer wait primitive, 2 waits each) on the same engine. Engine
    program order makes this equivalent: the stream blocks on the EVSEM
    waits, then on the instruction's remaining wait.
    """
    for bb in nc.main_func.blocks:
        new = []
        for ins in bb.instructions:
            w = (
                list(ins.sync_info.on_wait)
                if (ins.sync_info and ins.sync_info.on_wait)
                else []
            )
            if len(w) > 1:
                extras = w[:-1]
                for i in range(0, len(extras), 2):
                    ev = mybir.InstEventSemaphore(
                        name=nc.get_next_instruction_name(),
                        engine=ins.engine,
                        sync_info=mybir.SyncInfo(
                            on_wait=extras[i : i + 2], on_update=[]
                        ),
                    )
                    nc.register_instruction(ev)
                    new.append(ev)
                ins.sync_info.on_wait = [w[-1]]
            new.append(ins)
        bb.instructions[:] = new


def _group_plan(T):
    """DMA grouping: small leading groups to prime the pipeline, then big."""
    plan = []
    t = 0
    for n in (1, 1, 2, 4):
        if t >= T:
            break
        n = min(n, T - t)
        plan.append((t, t + n))
        t += n
    while t < T:
        n = min(STEADY_GROUP, T - t)
        plan.append((t, t + n))
        t += n
    return plan


def _build_program(T, GM):
    """Build the SPMD Bass program: T chunks of C nodes, GM local segments."""
    f32 = mybir.dt.float32
    bf16 = mybir.dt.bfloat16
    fp8 = mybir.dt.float8e3
    T4 = T * SUB
    XT = 2 * C  # transposed-x bytes per chunk per partition
    XN = SUB * (D + 1)  # natural-x + ones bytes
    OH = SUB * GM  # one-hot bytes
    CB = XT + XN + OH

    nc = bass.Bass(trn_type="TRN2")
    xp = nc.dram_tensor("xp", [128, T, CB], fp8, kind="ExternalInput")
    # byte-packed preamble, one DMA gating the pipeline start:
    # w1a|w1b|w2|pad2|mcf(f32: b1|bl unused|iota unused|dbi)
    NF = 1 + 1
    PRE = 516 + 4 * NF
    pre = nc.dram_tensor("pre", [128, PRE], fp8, kind="ExternalInput")
    out = nc.dram_tensor("out", [GM, D], f32, kind="ExternalOutput")

    Exp = mybir.ActivationFunctionType.Exp
    Tanh = mybir.ActivationFunctionType.Tanh

    plan = _group_plan(T)

    with tile.TileContext(nc) as tc:
        with (
            tc.tile_pool(name="const", bufs=1) as const,
            tc.tile_pool(name="thtp", bufs=3) as thtp,
            tc.tile_pool(name="owp", bufs=8) as owp,
            tc.tile_pool(name="etp", bufs=3) as etp,
            tc.tile_pool(name="outp", bufs=1) as outp,
            tc.tile_pool(name="hps", bufs=3, space="PSUM") as hps,
            tc.tile_pool(name="sps", bufs=2, space="PSUM") as sps,
            tc.tile_pool(name="accp", bufs=1, space="PSUM") as accp,
        ):
            xpg = [
                const.tile([128, t1 - t0, CB], fp8, name=f"xpg{gi}")
                for gi, (t0, t1) in enumerate(plan)
            ]
            gidx = []
            for gi, (t0, t1) in enumerate(plan):
                for lt in range(t1 - t0):
                    gidx.append((gi, lt))

            # all input loads on the sync HWDGE ring, in consumption order
            pres = const.tile([128, PRE], fp8)
            nc.sync.dma_start(out=pres, in_=pre[:, :])
            for gi, (t0, t1) in enumerate(plan):
                nc.sync.dma_start(out=xpg[gi], in_=xp[:, t0:t1])

            w1a = pres[:, 0:256].bitcast(bf16)
            w1b = pres[:, 256:512].bitcast(bf16)
            w2sb = pres[:, 512:514].bitcast(bf16)
            mcfv = pres[:, 516 : 516 + 4 * NF].bitcast(f32)
            b1sb = mcfv[:, 0:1]
            dbsb = mcfv[:, 1:2]

            def xt_half(c, h):
                gi, lt = gidx[c]
                return xpg[gi][:, lt, h * C : (h + 1) * C]

            def xn_sub(c, a):
                gi, lt = gidx[c]
                o = XT + a * (D + 1)
                return xpg[gi][:, lt, o : o + D + 1]

            def oh_chunk(c):
                gi, lt = gidx[c]
                return xpg[gi][:, lt, XT + XN : CB].rearrange(
                    "p (s g) -> p s g", s=SUB
                )

            # persistent PSUM accumulators; subtiles alternate banks so
            # back-to-back accumulate turnarounds overlap.
            pchA = accp.tile([GM, D + 1], f32)
            pchB = accp.tile([GM, D + 1], f32)

            NB = (T + Q - 1) // Q
            tht_t = [None] * T
            spb_t = [None] * NB
            etb_t = [None] * NB
            owt_t = [None] * T

            for j in range(T + Q + 1):
                # stage A: W1 matmul + tanh for chunk j
                if j < T:
                    if j % Q == 0:
                        spb_t[j // Q] = sps.tile([128, Q * SUB], f32, name="spb")
                    hp = hps.tile([H, C], f32)
                    nc.tensor.matmul(
                        hp, lhsT=w1a, rhs=xt_half(j, 0), start=True, stop=False
                    )
                    nc.tensor.matmul(
                        hp, lhsT=w1b, rhs=xt_half(j, 1), start=False, stop=True
                    )
                    tht = thtp.tile([H, C], bf16)
                    nc.scalar.activation(tht, hp, Tanh, bias=b1sb)
                    tht_t[j] = tht

                # stage B: score matmuls for chunk j-1 into its batch slot
                if 0 <= j - 1 < T:
                    jb = j - 1
                    b = jb // Q
                    spb = spb_t[b]
                    tht = tht_t[jb]
                    col0 = (jb % Q) * SUB
                    for a in range(SUB):
                        nc.tensor.matmul(
                            spb[:, col0 + a : col0 + a + 1],
                            lhsT=tht[:, a * 128 : (a + 1) * 128],
                            rhs=w2sb,
                            start=True,
                            stop=True,
                            skip_group_check=True,
                        )
                    # stage C: batch complete -> one exp, then one
                    # tensor_tensor one-hot build per chunk of the batch
                    if jb % Q == Q - 1 or jb == T - 1:
                        qc = (jb % Q) + 1
                        etb = etp.tile([128, Q * SUB], f32)
                        nc.scalar.activation(
                            etb[:, 0 : qc * SUB], spb[:, 0 : qc * SUB], Exp
                        )
                        etb_t[b] = etb
                        for c in range(b * Q, b * Q + qc):
                            owt = owp.tile([128, SUB, GM], bf16)
                            q0 = (c % Q) * SUB
                            ebc = (
                                etb[:, q0 : q0 + SUB]
                                .unsqueeze(2)
                                .broadcast_to([128, SUB, GM])
                            )
                            nc.vector.tensor_tensor(
                                out=owt,
                                in0=oh_chunk(c),
                                in1=ebc,
                                op=mybir.AluOpType.mult,
                            )
                            owt_t[c] = owt

                # stage E: pooling matmuls for chunk j-Q-1 (ow ready at
                # least one iteration ago, so the PE never waits here)
                jc = j - Q - 1
                if 0 <= jc < T:
                    owt = owt_t[jc]
                    for a in range(SUB):
                        pch = pchA if a % 2 == 0 else pchB
                        nc.tensor.matmul(
                            pch,
                            lhsT=owt[:, a, :],
                            rhs=xn_sub(jc, a),
                            start=(jc == 0 and a < 2),
                            stop=(jc == T - 1 and a >= 2),
                            skip_group_check=True,
                        )

            # combine banks + normalize:
            # out[g, :] = (A+B)[g, :D] / ((A+B)[g, D] + empty_guard)
            tsa = outp.tile([GM, D + 1], f32)
            nc.scalar.copy(tsa, pchA)
            ts = outp.tile([GM, D + 1], f32)
            nc.vector.tensor_add(ts, tsa, pchB)
            dn = outp.tile([GM, 1], f32)
            nc.vector.tensor_scalar_add(dn, ts[:, D : D + 1], dbsb[0:GM, 0:1])
            rc = outp.tile([GM, 1], f32)
            nc.vector.reciprocal(rc, dn)
            ot = outp.tile([GM, D], f32)
            nc.vector.tensor_scalar_mul(ot, ts[:, 0:D], rc[:, 0:1])
            # scalar HWDGE ring: empty at this point, while the sync ring
            # may still be retiring trailing input groups
            nc.scalar.dma_start(out=out[:, :], in_=ot)

    _split_multiwait(nc)
    return nc


def _prepare(inputs):
    """Host-side sharding and input staging. Returns (meta, in_maps)."""
    x = np.asarray(inputs["x"], dtype=np.float32)
    batch = np.asarray(inputs["batch"]).astype(np.int64)
    W1 = np.asarray(inputs["W1"], dtype=np.float32)
    b1 = np.asarray(inputs["b1"], dtype=np.float32)
    W2 = np.asarray(inputs["W2"], dtype=np.float32)

    n, d = x.shape
    assert d == D
    G = 512
    seg_ptr = np.searchsorted(batch, np.arange(G + 1))  # [G+1], seg g rows

    # split at segment boundaries, balancing rows
    targets = (np.arange(N_CORES + 1) * n) // N_CORES
    g_bounds = np.zeros(N_CORES + 1, dtype=np.int64)
    g_bounds[N_CORES] = G
    for c in range(1, N_CORES):
        g = int(np.argmin(np.abs(seg_ptr.astype(np.int64) - targets[c])))
        g_bounds[c] = max(g, g_bounds[c - 1])
    row_bounds = seg_ptr[g_bounds]

    rows = np.diff(row_bounds)
    segs = np.diff(g_bounds)
    GM = int(segs.max())
    assert GM <= 128, f"too many segments on one core: {GM}"
    T = int(-(-int(rows.max()) // C))
    R = T * C
    T4 = T * SUB
    XT = 2 * C
    XN = SUB * (D + 1)
    OH = SUB * GM

    # bf16 consts: W1 halves | W2, byte view for the packed preamble
    mcb = np.zeros((128, 2 * H + 1), dtype=BF16)
    mcb[:, 0:H] = W1[0:128].astype(BF16)
    mcb[:, H : 2 * H] = W1[128:256].astype(BF16)
    mcb[:, 2 * H] = W2[:, 0].astype(BF16)
    mcb_bytes = np.ascontiguousarray(mcb).view(np.uint8)  # [128, 514]

    in_maps = []
    for c in range(N_CORES):
        r0, r1 = int(row_bounds[c]), int(row_bounds[c + 1])
        g0, g1 = int(g_bounds[c]), int(g_bounds[c + 1])
        nr = r1 - r0
        xpad = np.zeros((R, D), dtype=np.float32)
        xpad[:nr] = x[r0:r1]
        xe = xpad.astype(E3M4)
        # transposed layout: [128, T, 2, C] fp8 e3m4
        xth = np.ascontiguousarray(xe.reshape(T, C, 2, 128).transpose(3, 0, 2, 1))
        # natural layout + ones column: [128, T, SUB, D+1] fp8
        xnb = np.empty((R, D + 1), dtype=E3M4)
        xnb[:, :D] = xe
        xnb[:, D] = np.float32(1.0)
        xnh = np.ascontiguousarray(
            xnb.reshape(T, SUB, 128, D + 1).transpose(2, 0, 1, 3)
        )
        # one-hot pattern (bl_i == g): [128, T, SUB, GM] fp8 {0, 1}
        bl = np.full(R, -1, dtype=np.int64)
        bl[:nr] = batch[r0:r1] - g0
        ohp = (
            bl.reshape(T, SUB, 128)[:, :, :, None]
            == np.arange(GM, dtype=np.int64)[None, None, None, :]
        ).astype(E3M4)
        ohh = np.ascontiguousarray(ohp.transpose(2, 0, 1, 3))
        # packed per-chunk stream: [128, T, XT+XN+OH]
        xph = np.concatenate(
            [
                xth.reshape(128, T, XT),
                xnh.reshape(128, T, XN),
                ohh.reshape(128, T, OH),
            ],
            axis=2,
        )
        # 1.0 guard for empty or padded segments (their denominator is 0)
        seg_count = np.zeros(GM, dtype=np.int64)
        cnts = seg_ptr[g0 + 1 : g1 + 1] - seg_ptr[g0:g1]
        seg_count[: g1 - g0] = cnts
        mcf = np.zeros((128, 2), dtype=np.float32)
        mcf[:, 0] = b1
        mcf[:GM, 1] = (seg_count == 0).astype(np.float32)
        pre = np.concatenate(
            [mcb_bytes, np.zeros((128, 2), dtype=np.uint8),
             np.ascontiguousarray(mcf).view(np.uint8)],
            axis=1,
        ).view(E3M4)
        in_maps.append({"xp": xph, "pre": pre})

    meta = {
        "T": T,
        "GM": GM,
        "g_bounds": g_bounds,
        "G": G,
        "n": n,
    }
    return meta, in_maps


def _gather(meta, res):
    G = meta["G"]
    g_bounds = meta["g_bounds"]
    full = np.zeros((G, D), dtype=np.float32)
    for c in range(N_CORES):
        g0, g1 = int(g_bounds[c]), int(g_bounds[c + 1])
        if g1 > g0:
            full[g0:g1] = res.results[c]["out"][: g1 - g0]
    return full


def _sane(full):
    # output rows are convex combinations of x rows (|x| < ~6); a device
    # glitch shows up as a huge value or NaN.
    return bool(np.isfinite(full).all() and np.abs(full).max() < 64.0)


def _run(inputs, trace=False):
    meta, in_maps = _prepare(inputs)
    nc = _build_program(meta["T"], meta["GM"])
    try:
        res = run_bass_kernel_spmd(nc, in_maps, list(range(N_CORES)), trace=trace)
        full = _gather(meta, res)
        if not _sane(full):
            raise RuntimeError("insane output, retrying once")
    except Exception:
        # transient device failures (e.g. NRT_EXEC_UNIT_UNRECOVERABLE) happen;
        # one rebuild+retry
        nc = _build_program(meta["T"], meta["GM"])
        res = run_bass_kernel_spmd(nc, in_maps, list(range(N_CORES)), trace=trace)
        full = _gather(meta, res)
    return full, res


def kernel(**inputs) -> np.ndarray:
    out, _ = _run(inputs, trace=False)
    return out


def kernel_traced(**inputs):
    """Returns (output, BassKernelResults with exec_time_ns/profile)."""
    out, res = _run(inputs, trace=True)
    return out, res


# revision 16
# speedup vs baseline: 2.1862x; 1.1861x over previous
"""AttentionPooling (segment softmax-pool) Trainium2 Bass kernel, v3.

out[g, :] = sum_{i: batch[i]==g} softmax_within_segment(score)_i * x[i, :]
score_i = tanh(x_i @ W1 + b1) @ W2 + b2

Math notes:
- softmax is shift-invariant, so b2 and the per-segment max subtraction
  cancel exactly; we compute e_i = exp(s_i) with s_i = tanh(xW1+b1)@W2
  and normalize by the per-segment sum of e at the very end (column D of
  the PSUM accumulator, via the ones column appended to x).
- per-segment sums run on the TensorEngine: for each 128-node subtile,
  the one-hot matrix ow[i, g] = e_i * (batch_local[i] == g) is the
  stationary operand and [x | 1] (fp8) the moving one; fp8 moving double
  pumps on HW (~0.21 ns/row), so each 257-row pool matmul is ~120 ns.
  Accumulation alternates between TWO psum banks (pchA even subtiles,
  pchB odd) to hide the per-bank accumulate turnaround; the banks are
  summed at the end.

Precision: both x copies ride fp8 e3m4 (the score MLP consumes the
transposed copy, pooling the natural copy + ones column). Validated
rel_err ~1.5e-2 vs the f32 reference (gate 2e-2).

Engine-level changes vs v1 (82.5us):
- DMA: 19.9 MB -> 14.8 MB (natural copy fp8 instead of bf16, plus a
  preloaded fp8 one-hot pattern per subtile).
- DVE: the 4 tensor_scalar one-hot builds per chunk (~300 ns each,
  instruction-overhead-bound) become ONE tensor_tensor: preloaded
  one-hot (bl_i == g) times exp(s) broadcast via a stride-0 AP.
- ACT: exp is batched over Q=4 chunks into one [128,16] instruction.
- PE: pool matmuls consume fp8 (2x moving rate).

Pipelining: stage skew keeps cross-engine inputs produced at least one
full chunk ahead: W1/tanh(j), scores(j-1), exp+ow at batch boundaries,
pool(j-Q-1).

HBM traffic per core: one packed fp8 stream [128, T, 2312] (per chunk
and partition: 1024B transposed x | 1028B natural x+ones | 260B
one-hot), grouped DMAs on the sync HWDGE ring in consumption order. A
byte-packed preamble DMA carries the weights/consts so a single
completion gates pipeline start.

Sharding: nodes split across 8 cores at segment boundaries (batch is
sorted); each core reduces its own segments; host concatenates the
per-core [G_c, D] outputs.
"""

import sys

sys.path.insert(0, "/opt/trn_rl_repo")

import numpy as np
import ml_dtypes

import concourse.bass as bass
import concourse.tile as tile
from concourse import mybir
from concourse.bass_utils import run_bass_kernel_spmd

BF16 = ml_dtypes.bfloat16
E3M4 = ml_dtypes.float8_e3m4

N_CORES = 8
D = 256
H = 128  # hidden dim of the score MLP
C = 512  # nodes per chunk
SUB = C // 128
Q = 4  # chunks per exp batch
STEADY_GROUP = 6  # chunks per steady-state DMA group


def _split_multiwait(nc):
    """Split multi-wait instructions for this walrus build.

    This neuronxcc/walrus rejects more than one sync-wait command per
    instruction ("Too many sync wait commands"), but tile emits 2-3 waits
    on compute/DMA instructions and many on the final Drain. Hoist the
    extra waits onto preceding InstEventSemaphore instructions (the native
    sequencer wait primitive, 2 waits each) on the same engine. Engine
    program order makes this equivalent: the stream blocks on the EVSEM
    waits, then on the instruction's remaining wait.
    """
    for bb in nc.main_func.blocks:
        new = []
        for ins in bb.instructions:
            w = (
                list(ins.sync_info.on_wait)
                if (ins.sync_info and ins.sync_info.on_wait)
                else []
            )
            if len(w) > 1:
                extras = w[:-1]
                for i in range(0, len(extras), 2):
                    ev = mybir.InstEventSemaphore(
                        name=nc.get_next_instruction_name(),
                        engine=ins.engine,
                        sync_info=mybir.SyncInfo(
                            on_wait=extras[i : i + 2], on_update=[]
                        ),
                    )
                    nc.register_instruction(ev)
                    new.append(ev)
                ins.sync_info.on_wait = [w[-1]]
            new.append(ins)
        bb.instructions[:] = new


def _group_plan(T):
    """DMA grouping: small leading groups to prime the pipeline, then big."""
    plan = []
    t = 0
    for n in (1, 1, 2, 4):
        if t >= T:
            break
        n = min(n, T - t)
        plan.append((t, t + n))
        t += n
    while t < T:
        n = min(STEADY_GROUP, T - t)
        plan.append((t, t + n))
        t += n
    return plan


def _build_program(T, GM):
    """Build the SPMD Bass program: T chunks of C nodes, GM local segments."""
    f32 = mybir.dt.float32
    bf16 = mybir.dt.bfloat16
    fp8 = mybir.dt.float8e3
    T4 = T * SUB
    XT = 2 * C  # transposed-x bytes per chunk per partition
    XS = D + 4  # natural-x subtile slot: x | 1 | 0 | pad, 4-byte aligned
    XN = SUB * XS  # natural-x + ones bytes
    OH = SUB * GM  # one-hot bytes
    CB = XT + XN + OH

    nc = bass.Bass(trn_type="TRN2")
    xp = nc.dram_tensor("xp", [128, T, CB], fp8, kind="ExternalInput")
    # byte-packed preamble, one DMA gating the pipeline start:
    # w1a|w1b|w2|pad2|mcf(f32: b1|bl unused|iota unused|dbi)
    NF = 1 + 1
    PRE = 516 + 4 * NF
    pre = nc.dram_tensor("pre", [128, PRE], fp8, kind="ExternalInput")
    out = nc.dram_tensor("out", [GM, D], f32, kind="ExternalOutput")

    Exp = mybir.ActivationFunctionType.Exp
    Tanh = mybir.ActivationFunctionType.Tanh

    plan = _group_plan(T)

    with tile.TileContext(nc) as tc:
        with (
            tc.tile_pool(name="const", bufs=1) as const,
            tc.tile_pool(name="thtp", bufs=3) as thtp,
            tc.tile_pool(name="owp", bufs=8) as owp,
            tc.tile_pool(name="etp", bufs=3) as etp,
            tc.tile_pool(name="outp", bufs=1) as outp,
            tc.tile_pool(name="hps", bufs=3, space="PSUM") as hps,
            tc.tile_pool(name="sps", bufs=2, space="PSUM") as sps,
            tc.tile_pool(name="accp", bufs=1, space="PSUM") as accp,
        ):
            xpg = [
                const.tile([128, t1 - t0, CB], fp8, name=f"xpg{gi}")
                for gi, (t0, t1) in enumerate(plan)
            ]
            gidx = []
            for gi, (t0, t1) in enumerate(plan):
                for lt in range(t1 - t0):
                    gidx.append((gi, lt))

            # all input loads on the sync HWDGE ring, in consumption order
            pres = const.tile([128, PRE], fp8)
            nc.sync.dma_start(out=pres, in_=pre[:, :])
            for gi, (t0, t1) in enumerate(plan):
                nc.sync.dma_start(out=xpg[gi], in_=xp[:, t0:t1])

            w1a = pres[:, 0:256].bitcast(bf16)
            w1b = pres[:, 256:512].bitcast(bf16)
            w2sb = pres[:, 512:514].bitcast(bf16)
            mcfv = pres[:, 516 : 516 + 4 * NF].bitcast(f32)
            b1sb = mcfv[:, 0:1]
            dbsb = mcfv[:, 1:2]

            def xt_half(c, h):
                gi, lt = gidx[c]
                return xpg[gi][:, lt, h * C : (h + 1) * C]

            def xn_sub(c, a):
                # 258 moving rows (x | 1 | 0): even row count + 4-byte
                # aligned base unlock the fp8 moving double-pump
                gi, lt = gidx[c]
                o = XT + a * XS
                return xpg[gi][:, lt, o : o + D + 2]

            def oh_chunk(c):
                gi, lt = gidx[c]
                return xpg[gi][:, lt, XT + XN : CB].rearrange(
                    "p (s g) -> p s g", s=SUB
                )

            # persistent PSUM accumulators; subtiles alternate banks so
            # back-to-back accumulate turnarounds overlap.
            pchA = accp.tile([GM, D + 2], f32)
            pchB = accp.tile([GM, D + 2], f32)

            NB = (T + Q - 1) // Q
            tht_t = [None] * T
            spb_t = [None] * NB
            etb_t = [None] * NB
            owt_t = [None] * T

            for j in range(T + Q + 1):
                # stage A: W1 matmul + tanh for chunk j
                if j < T:
                    if j % Q == 0:
                        spb_t[j // Q] = sps.tile([128, Q * SUB], f32, name="spb")
                    hp = hps.tile([H, C], f32)
                    nc.tensor.matmul(
                        hp, lhsT=w1a, rhs=xt_half(j, 0), start=True, stop=False
                    )
                    nc.tensor.matmul(
                        hp, lhsT=w1b, rhs=xt_half(j, 1), start=False, stop=True
                    )
                    tht = thtp.tile([H, C], bf16)
                    nc.scalar.activation(tht, hp, Tanh, bias=b1sb)
                    tht_t[j] = tht

                # stage B: score matmuls for chunk j-1 into its batch slot
                if 0 <= j - 1 < T:
                    jb = j - 1
                    b = jb // Q
                    spb = spb_t[b]
                    tht = tht_t[jb]
                    col0 = (jb % Q) * SUB
                    for a in range(SUB):
                        nc.tensor.matmul(
                            spb[:, col0 + a : col0 + a + 1],
                            lhsT=tht[:, a * 128 : (a + 1) * 128],
                            rhs=w2sb,
                            start=True,
                            stop=True,
                            skip_group_check=True,
                        )
                    # stage C: batch complete -> one exp, then one
                    # tensor_tensor one-hot build per chunk of the batch
                    if jb % Q == Q - 1 or jb == T - 1:
                        qc = (jb % Q) + 1
                        etb = etp.tile([128, Q * SUB], f32)
                        nc.scalar.activation(
                            etb[:, 0 : qc * SUB], spb[:, 0 : qc * SUB], Exp
                        )
                        etb_t[b] = etb
                        for c in range(b * Q, b * Q + qc):
                            owt = owp.tile([128, SUB, GM], bf16)
                            q0 = (c % Q) * SUB
                            ebc = (
                                etb[:, q0 : q0 + SUB]
                                .unsqueeze(2)
                                .broadcast_to([128, SUB, GM])
                            )
                            nc.vector.tensor_tensor(
                                out=owt,
                                in0=oh_chunk(c),
                                in1=ebc,
                                op=mybir.AluOpType.mult,
                            )
                            owt_t[c] = owt

                # stage E: pooling matmuls for chunk j-Q-1 (ow ready at
                # least one iteration ago, so the PE never waits here)
                jc = j - Q - 1
                if 0 <= jc < T:
                    owt = owt_t[jc]
                    for a in range(SUB):
                        pch = pchA if a % 2 == 0 else pchB
                        nc.tensor.matmul(
                            pch,
                            lhsT=owt[:, a, :],
                            rhs=xn_sub(jc, a),
                            start=(jc == 0 and a < 2),
                            stop=(jc == T - 1 and a >= 2),
                            skip_group_check=True,
                        )

            # combine banks + normalize:
            # out[g, :] = (A+B)[g, :D] / ((A+B)[g, D] + empty_guard)
            tsa = outp.tile([GM, D + 2], f32)
            nc.scalar.copy(tsa, pchA)
            ts = outp.tile([GM, D + 2], f32)
            nc.vector.tensor_add(ts, tsa, pchB)
            dn = outp.tile([GM, 1], f32)
            nc.vector.tensor_scalar_add(dn, ts[:, D : D + 1], dbsb[0:GM, 0:1])
            rc = outp.tile([GM, 1], f32)
            nc.vector.reciprocal(rc, dn)
            ot = outp.tile([GM, D], f32)
            nc.vector.tensor_scalar_mul(ot, ts[:, 0:D], rc[:, 0:1])
            # scalar HWDGE ring: empty at this point, while the sync ring
            # may still be retiring trailing input groups
            nc.scalar.dma_start(out=out[:, :], in_=ot)

    _split_multiwait(nc)
    return nc


def _prepare(inputs):
    """Host-side sharding and input staging. Returns (meta, in_maps)."""
    x = np.asarray(inputs["x"], dtype=np.float32)
    batch = np.asarray(inputs["batch"]).astype(np.int64)
    W1 = np.asarray(inputs["W1"], dtype=np.float32)
    b1 = np.asarray(inputs["b1"], dtype=np.float32)
    W2 = np.asarray(inputs["W2"], dtype=np.float32)

    n, d = x.shape
    assert d == D
    G = 512
    seg_ptr = np.searchsorted(batch, np.arange(G + 1))  # [G+1], seg g rows

    # split at segment boundaries, balancing rows
    targets = (np.arange(N_CORES + 1) * n) // N_CORES
    g_bounds = np.zeros(N_CORES + 1, dtype=np.int64)
    g_bounds[N_CORES] = G
    for c in range(1, N_CORES):
        g = int(np.argmin(np.abs(seg_ptr.astype(np.int64) - targets[c])))
        g_bounds[c] = max(g, g_bounds[c - 1])
    row_bounds = seg_ptr[g_bounds]

    rows = np.diff(row_bounds)
    segs = np.diff(g_bounds)
    GM = int(segs.max())
    assert GM <= 128, f"too many segments on one core: {GM}"
    T = int(-(-int(rows.max()) // C))
    R = T * C
    T4 = T * SUB
    XT = 2 * C
    XS = D + 4
    XN = SUB * XS
    OH = SUB * GM

    # bf16 consts: W1 halves | W2, byte view for the packed preamble
    mcb = np.zeros((128, 2 * H + 1), dtype=BF16)
    mcb[:, 0:H] = W1[0:128].astype(BF16)
    mcb[:, H : 2 * H] = W1[128:256].astype(BF16)
    mcb[:, 2 * H] = W2[:, 0].astype(BF16)
    mcb_bytes = np.ascontiguousarray(mcb).view(np.uint8)  # [128, 514]

    in_maps = []
    for c in range(N_CORES):
        r0, r1 = int(row_bounds[c]), int(row_bounds[c + 1])
        g0, g1 = int(g_bounds[c]), int(g_bounds[c + 1])
        nr = r1 - r0
        xpad = np.zeros((R, D), dtype=np.float32)
        xpad[:nr] = x[r0:r1]
        xe = xpad.astype(E3M4)
        # transposed layout: [128, T, 2, C] fp8 e3m4
        xth = np.ascontiguousarray(xe.reshape(T, C, 2, 128).transpose(3, 0, 2, 1))
        # natural layout + ones column, padded to a 4-byte-aligned
        # 260-byte slot: [128, T, SUB, XS] fp8
        xnb = np.zeros((R, XS), dtype=E3M4)
        xnb[:, :D] = xe
        xnb[:, D] = np.float32(1.0)
        xnh = np.ascontiguousarray(
            xnb.reshape(T, SUB, 128, XS).transpose(2, 0, 1, 3)
        )
        # one-hot pattern (bl_i == g): [128, T, SUB, GM] fp8 {0, 1}
        bl = np.full(R, -1, dtype=np.int64)
        bl[:nr] = batch[r0:r1] - g0
        ohp = (
            bl.reshape(T, SUB, 128)[:, :, :, None]
            == np.arange(GM, dtype=np.int64)[None, None, None, :]
        ).astype(E3M4)
        ohh = np.ascontiguousarray(ohp.transpose(2, 0, 1, 3))
        # packed per-chunk stream: [128, T, XT+XN+OH]
        xph = np.concatenate(
            [
                xth.reshape(128, T, XT),
                xnh.reshape(128, T, XN),
                ohh.reshape(128, T, OH),
            ],
            axis=2,
        )
        # 1.0 guard for empty or padded segments (their denominator is 0)
        seg_count = np.zeros(GM, dtype=np.int64)
        cnts = seg_ptr[g0 + 1 : g1 + 1] - seg_ptr[g0:g1]
        seg_count[: g1 - g0] = cnts
        mcf = np.zeros((128, 2), dtype=np.float32)
        mcf[:, 0] = b1
        mcf[:GM, 1] = (seg_count == 0).astype(np.float32)
        pre = np.concatenate(
            [mcb_bytes, np.zeros((128, 2), dtype=np.uint8),
             np.ascontiguousarray(mcf).view(np.uint8)],
            axis=1,
        ).view(E3M4)
        in_maps.append({"xp": xph, "pre": pre})

    meta = {
        "T": T,
        "GM": GM,
        "g_bounds": g_bounds,
        "G": G,
        "n": n,
    }
    return meta, in_maps


def _gather(meta, res):
    G = meta["G"]
    g_bounds = meta["g_bounds"]
    full = np.zeros((G, D), dtype=np.float32)
    for c in range(N_CORES):
        g0, g1 = int(g_bounds[c]), int(g_bounds[c + 1])
        if g1 > g0:
            full[g0:g1] = res.results[c]["out"][: g1 - g0]
    return full


def _sane(full):
    # output rows are convex combinations of x rows (|x| < ~6); a device
    # glitch shows up as a huge value or NaN.
    return bool(np.isfinite(full).all() and np.abs(full).max() < 64.0)


def _run(inputs, trace=False):
    meta, in_maps = _prepare(inputs)
    nc = _build_program(meta["T"], meta["GM"])
    try:
        res = run_bass_kernel_spmd(nc, in_maps, list(range(N_CORES)), trace=trace)
        full = _gather(meta, res)
        if not _sane(full):
            raise RuntimeError("insane output, retrying once")
    except Exception:
        # transient device failures (e.g. NRT_EXEC_UNIT_UNRECOVERABLE) happen;
        # one rebuild+retry
        nc = _build_program(meta["T"], meta["GM"])
        res = run_bass_kernel_spmd(nc, in_maps, list(range(N_CORES)), trace=trace)
        full = _gather(meta, res)
    return full, res


def kernel(**inputs) -> np.ndarray:
    out, _ = _run(inputs, trace=False)
    return out


def kernel_traced(**inputs):
    """Returns (output, BassKernelResults with exec_time_ns/profile)."""
    out, res = _run(inputs, trace=True)
    return out, res
